# revision 44
# baseline (speedup 1.0000x reference)
"""Trainium2 Bass kernel for MemoryEfficientISNE GNN message passing (v3).

Full inputs in, full output out. 8-way data-parallel over nodes with a
balanced node->(core, bucket, slot) permutation computed on the host.

v3 design (over v2):
  - Host pre-adds node_features + emb_table[node_ids] (one xT input).
  - Host folds the per-layer LN mean into the weights (W' = W - rs @ 1^T),
    so ps_y = y - mean(y) comes straight out of the matmul accumulation.
  - LN scale invariance: LN((y-mu)*c) == LN(y-mu) for c>0, and
    relu(v*c) == relu(v)*c, so layers 0/1 skip the 1/std scaling entirely
    (it washes out in the next LN); only layer 2 applies rsig, computed as
    exp(-0.5*ln(var+eps)) on ACT (no DVE reciprocal, no table switches:
    everything in Phase A uses the natural_log_exp_and_others set).
  - Phase A emits per-node 576B rows [z bf16 x256 | t | pad] to chunked
    ag_in tensors, and 2*z f32 node-major into a persistent SBUF
    accumulator (agg_sb) -- no zN DRAM round trip.
  - Chunked AllGather (cfg.n_ag_chunks) on bucket-group boundaries.
  - Phase B: per destination-bucket pair, one merged dma_gather per half
    (1280 idxs). Attention is computed per-SLOT, not per (slot,dst):
    s[dst-of-slot] comes from 10 tiny matmuls against a transposed one-hot
    (oh2), so the sigmoid is [128,10] instead of 10x[128,128]. oat =
    oh * att scales the one-hot; 10 accumulating matmuls aggregate into
    [dst,256] PSUM; y = ps + agg_sb is stored back to SBUF with bn_stats.
  - Pass 2 batches all final-LN normalization (one Ln/Exp + per-bucket
    affine apply) so sigmoid/ln tables never interleave.

Self-contained: hardcodes the problem shapes from the task spec.
"""
from dataclasses import dataclass, replace

import numpy as np

import concourse.bacc as bacc
import concourse.bass as bass
import concourse.tile as tile
from concourse import mybir
from concourse.bass_utils import run_bass_kernel_spmd
from concourse.masks import make_identity

f32 = mybir.dt.float32
f32r = mybir.dt.float32r
bf16 = mybir.dt.bfloat16
i16 = mybir.dt.int16
AF = mybir.ActivationFunctionType
ALU = mybir.AluOpType

LN_EPS = 1e-5
P = 128


@dataclass(frozen=True)
class Cfg:
    n_cores: int = 8
    d_in: int = 256          # D
    d_hid: int = 512         # H
    shard: int = 6400        # padded nodes per core (= buckets * 128)
    cpb: int = 10            # gather chunks per bucket (5 lo + 5 hi)
    row: int = 384           # ag row in bf16 elems (768B; gather needs %256B)
    g_bufs: int = 3          # gather pair-buffers in SBUF (x2 tiles each)
    trace: bool = False
    b_att: float = 0.0
    n_ag_chunks: int = 2

    @property
    def buckets(self):
        return self.shard // P

    @property
    def totn(self):
        return self.n_cores * self.shard

    @property
    def half_slots(self):
        return (self.cpb // 2) * P

    # AllGather chunking: global row of (core c, slot s in chunk k) =
    # chunk_row_base[k] + c*chunk_rows[k] + (s - 128*bucket_base[k]).
    @property
    def bucket_base(self):
        B = self.buckets
        if self.n_ag_chunks == 1:
            return [0, B]
        if self.n_ag_chunks == 2:
            return [0, B // 2, B]
        return [0, 13, B // 2, 38, B]

    @property
    def chunk_rows(self):
        bb = self.bucket_base
        return [(bb[k + 1] - bb[k]) * P for k in range(len(bb) - 1)]

    @property
    def chunk_row_base(self):
        out = [0]
        for r in self.chunk_rows:
            out.append(out[-1] + r * self.n_cores)
        return out

    @property
    def half_rows(self):
        # both halves must stay < 32768 rows for int16 gather indices
        if self.n_ag_chunks == 1:
            return self.totn // 2
        return self.chunk_row_base[len(self.bucket_base) // 2]


CFG = Cfg()


def build(cfg: Cfg, cnts=None):
    nc = bacc.Bacc("TRN2", target_bir_lowering=False, debug=False,
                   num_devices=cfg.n_cores, num_swdge_queues=2)
    D, H, S = cfg.d_in, cfg.d_hid, cfg.shard
    B, CPB, R = cfg.buckets, cfg.cpb, cfg.row
    KD, KH = D // P, H // P
    NP = B // 2                      # gather pairs
    IW = (2 * cfg.half_slots) // 16  # idx cols per (pair, half) = 80

    # ---- I/O ----
    xT = nc.dram_tensor("xT", [D, S], f32, kind="ExternalInput").ap()
    w_in = [nc.dram_tensor(f"w{i}", shp, f32, kind="ExternalInput").ap()
            for i, shp in enumerate([[D, H], [H, H], [H, H], [H, D]])]
    watt = nc.dram_tensor("watt", [H, 2], f32, kind="ExternalInput").ap()
    dgidx = nc.dram_tensor("dgidx", [P, NP * 2 * IW], i16,
                           kind="ExternalInput").ap()
    ohT = nc.dram_tensor("ohT", [P, B * CPB * P], bf16,
                         kind="ExternalInput").ap()
    oh2T = nc.dram_tensor("oh2T", [P, B * CPB * P], bf16,
                          kind="ExternalInput").ap()

    outN = nc.dram_tensor("outN", [S, D], f32, kind="ExternalOutput").ap()

    # ---- internal DRAM ----
    bb, crows, crb = cfg.bucket_base, cfg.chunk_rows, cfg.chunk_row_base
    nch = len(bb) - 1
    assert nch == 2
    ag_in = [nc.dram_tensor(f"ag_in{k}", [crows[k], R], bf16, kind="Internal").ap()
             for k in range(nch)]
    ag_half = [nc.dram_tensor(
        f"ag_out{k}", [crb[k + 1] - crb[k], R], bf16, kind="Internal",
        addr_space="Shared").ap() for k in range(nch)]

    with tile.TileContext(nc) as tc:
        with (
            tc.tile_pool(name="consts", bufs=1) as consts,
            tc.tile_pool(name="wstage", bufs=2) as wstage,
        ):
            def load_w(src_ap, shape, name):
                t_f = wstage.tile([P, 512], f32, tag="wstage")
                nc.sync.dma_start(t_f[:shape[0], :shape[1]], src_ap)
                t_r = consts.tile(shape, f32r, tag=name)
                nc.vector.tensor_copy(t_r[:], t_f[:shape[0], :shape[1]])
                return t_r

            w_r = []
            for i, w in enumerate(w_in):
                kin = w.shape[0] // P
                w_r.append([load_w(w[kt * P:(kt + 1) * P, :], [P, w.shape[1]],
                                   f"w{i}_{kt}") for kt in range(kin)])
            wattr = [load_w(watt[kt * P:(kt + 1) * P, :], [P, 2], f"watt_{kt}")
                     for kt in range(KH)]

            ones_f = consts.tile([P, P], f32, tag="ones_f")
            nc.vector.memset(ones_f[:], 1.0)
            ones_row = consts.tile([1, P], f32r, tag="ones_row")
            nc.vector.tensor_copy(ones_row[:], ones_f[:1, :])
            ones_12r = consts.tile([1, 2], f32r, tag="ones_12r")
            nc.vector.tensor_copy(ones_12r[:], ones_f[:1, :2])
            ones_colbf = consts.tile([P, 1], bf16, tag="ones_colbf")
            nc.vector.memset(ones_colbf[:], 1.0)
            ones_1bf = consts.tile([1, 2], bf16, tag="ones_1bf")
            nc.vector.memset(ones_1bf[:], 1.0)

            ident_f = consts.tile([P, P], f32, tag="ident_f")
            make_identity(nc, ident_f[:])
            ident_r = consts.tile([P, P], f32r, tag="ident_r")
            nc.vector.tensor_copy(ident_r[:], ident_f[:])
            ident_bf = consts.tile([P, P], bf16, tag="ident_bf")
            nc.vector.tensor_copy(ident_bf[:], ident_f[:])

            idx_sb = consts.tile([P, NP * 2 * IW], i16, tag="idx_sb")
            nc.sync.dma_start(idx_sb[:], dgidx)

            eps_t = consts.tile([1, 1], f32, tag="eps_t")
            nc.vector.memset(eps_t[:], LN_EPS)
            tiny_t = consts.tile([1, 1], f32, tag="tiny_t")
            nc.vector.memset(tiny_t[:], 5e-18)
            lnh_t = consts.tile([1, 1], f32, tag="lnh_t")
            nc.vector.memset(lnh_t[:], 0.5 * float(np.log(512.0)))
            eps_col = consts.tile([P, 1], f32, tag="eps_col")
            nc.vector.memset(eps_col[:], LN_EPS)
            batt_sb = consts.tile([1, 1], f32, tag="batt_sb")
            nc.vector.memset(batt_sb[:], float(cfg.b_att))

            s_row = consts.tile([1, S], bf16, tag="s_row")
            agg_sb = consts.tile([P, B * D], f32, tag="agg_sb")
            mu_sb = consts.tile([P, B], f32, tag="mu_sb")
            var_sb = consts.tile([P, B], f32, tag="var_sb")

            # ======================= Phase A =======================
            with (
                tc.tile_pool(name="pa_sb", bufs=2) as pa_sb,
                tc.tile_pool(name="pa_sb1", bufs=1) as pa_sb1,
                tc.tile_pool(name="pa_ps_y", bufs=2, space="PSUM") as pa_ps_y,
                tc.tile_pool(name="pa_ps_m", bufs=1, space="PSUM") as pa_ps_m,
                tc.tile_pool(name="pa_ps_rb", bufs=1, space="PSUM") as pa_ps_rb,
                tc.tile_pool(name="pa_ps_tr", bufs=2, space="PSUM") as pa_ps_tr,
            ):
                tok = []
                s0 = 0
                while s0 < S:
                    T = min(512, S - s0)
                    tok.append((s0, T))
                    s0 += T

                def mm_layer(x_tiles, W, T, n_out, tag):
                    """ps_y tiles for one layer; returns list of psum tiles."""
                    kin = len(x_tiles)
                    outs = []
                    for m in range(n_out // P):
                        ps_y = pa_ps_y.tile([P, 512], f32, tag="ps_y",
                                            name=f"ps_{tag}{m}")
                        for kt in range(kin):
                            nc.tensor.matmul(
                                ps_y[:, :T],
                                lhsT=W[kt][:, m * P:(m + 1) * P],
                                rhs=x_tiles[kt][:, :T],
                                start=(kt == 0), stop=(kt == kin - 1))
                        outs.append(ps_y)
                    return outs

                for (s0, T) in tok:
                    x0 = []
                    for kt in range(KD):
                        x_f = pa_sb.tile([P, 512], f32, tag=f"x0f_{kt}")
                        nc.sync.dma_start(x_f[:, :T],
                                          xT[kt * P:(kt + 1) * P, s0:s0 + T])
                        x_t = pa_sb.tile([P, 512], f32r, tag=f"x0_{kt}")
                        nc.vector.tensor_copy(x_t[:, :T], x_f[:, :T])
                        x0.append(x_t)

                    # L0: x1 = relu(y0 - mu0)  (ACT), no scaling
                    x1 = []
                    for m, ps in enumerate(mm_layer(x0, w_r[0], T, H, "y0")):
                        x_t = pa_sb1.tile([P, 512], f32r, tag=f"x1_{m}")
                        nc.scalar.activation(x_t[:, :T], ps[:, :T], AF.Relu)
                        x1.append(x_t)

                    # L1: x2 = relu(y1 - mu1)  (DVE), no scaling; also track
                    # ssq(y1) for the eps compensation of LN2
                    x2, sq1 = [], []
                    for m, ps in enumerate(mm_layer(x1, w_r[1], T, H, "y1")):
                        x_t = pa_sb1.tile([P, 512], f32r, tag=f"x2_{m}")
                        nc.vector.tensor_scalar(out=x_t[:, :T], in0=ps[:, :T],
                                                scalar1=0.0, scalar2=None,
                                                op0=ALU.max)
                        x2.append(x_t)
                        sq_m = pa_sb1.tile([P, 512], bf16, tag=f"sq1_{m}")
                        nc.scalar.activation(sq_m[:, :T], ps[:, :T], AF.Square)
                        sq1.append(sq_m)
                    ps_ssq1 = pa_ps_m.tile([1, 512], f32, tag="ps_ssq",
                                           name="ps_ssq1")
                    for m in range(KH):
                        nc.tensor.matmul(ps_ssq1[:, :T],
                                         lhsT=ones_colbf[:],
                                         rhs=sq1[m][:, :T],
                                         start=(m == 0), stop=(m == KH - 1))
                    u_row = pa_sb1.tile([1, 512], f32, tag="u_row")
                    nc.vector.tensor_scalar(out=u_row[:, :T],
                                            in0=ps_ssq1[:, :T],
                                            scalar1=LN_EPS, scalar2=None,
                                            op0=ALU.mult)

                    # L2: full LN (sq on ACT, rl on DVE)
                    sq, rl = [], []
                    for m, ps in enumerate(mm_layer(x2, w_r[2], T, H, "y2")):
                        sq_m = pa_sb1.tile([P, 512], bf16, tag=f"sq{m}")
                        nc.scalar.activation(sq_m[:, :T], ps[:, :T], AF.Square)
                        sq.append(sq_m)
                        rl_m = pa_sb1.tile([P, 512], f32r, tag=f"rl{m}")
                        nc.vector.tensor_scalar(out=rl_m[:, :T], in0=ps[:, :T],
                                                scalar1=0.0, scalar2=None,
                                                op0=ALU.max)
                        rl.append(rl_m)

                    ps_ssq = pa_ps_m.tile([1, 512], f32, tag="ps_ssq",
                                          name="ps_ssq2")
                    for m in range(KH):
                        nc.tensor.matmul(ps_ssq[:, :T],
                                         lhsT=ones_colbf[:],
                                         rhs=sq[m][:, :T],
                                         start=(m == 0), stop=(m == KH - 1))
                    # w = ssq2 + eps*ssq1;  rsig = sqrt(H) * w^-0.5
                    wsum = pa_sb1.tile([1, 512], f32, tag="wsum")
                    nc.vector.tensor_add(wsum[:, :T], ps_ssq[:, :T],
                                         u_row[:, :T])
                    lnv = pa_sb1.tile([1, 512], f32, tag="lnv")
                    nc.scalar.activation(lnv[:, :T], wsum[:, :T], AF.Ln,
                                         bias=tiny_t[:, :1])
                    rsig = pa_sb1.tile([1, 512], f32r, tag="rsig")
                    nc.scalar.activation(rsig[:, :T], lnv[:, :T], AF.Exp,
                                         scale=-0.5, bias=lnh_t[:, :1])
                    ps_rb = pa_ps_rb.tile([P, 512], f32, tag="ps_rb")
                    nc.tensor.matmul(ps_rb[:, :T], lhsT=ones_row[:],
                                     rhs=rsig[:, :T],
                                     start=True, stop=True)

                    h2 = []
                    for m in range(KH):
                        o_m = pa_sb1.tile([P, 512], f32r, tag=f"h2_{m}")
                        nc.vector.tensor_mul(o_m[:, :T], rl[m][:, :T],
                                             ps_rb[:, :T])
                        h2.append(o_m)

                    ps_s = pa_ps_m.tile([1, 512], f32, tag="ps_st",
                                        name="ps_s")
                    for kt in range(KH):
                        nc.tensor.matmul(ps_s[:, :T],
                                         lhsT=wattr[kt][:, 0:1],
                                         rhs=h2[kt][:, :T],
                                         start=(kt == 0), stop=(kt == KH - 1))
                    nc.scalar.activation(s_row[:, s0:s0 + T], ps_s[:, :T],
                                         AF.Identity, bias=batt_sb[:, :1])
                    ps_t = pa_ps_m.tile([1, 512], f32, tag="ps_st",
                                        name="ps_t")
                    for kt in range(KH):
                        nc.tensor.matmul(ps_t[:, :T],
                                         lhsT=wattr[kt][:, 1:2],
                                         rhs=h2[kt][:, :T],
                                         start=(kt == 0), stop=(kt == KH - 1))
                    t_row = pa_sb1.tile([1, 512], f32r, tag="t_row")
                    nc.vector.tensor_copy(t_row[:, :T], ps_t[:, :T])

                    # z = h2 @ W3 (dim-major)
                    zsb = []
                    for m in range(KD):
                        ps_z = pa_ps_y.tile([P, 512], f32, tag="ps_y",
                                            name=f"ps_z{m}")
                        for kt in range(KH):
                            nc.tensor.matmul(
                                ps_z[:, :T],
                                lhsT=w_r[3][kt][:, m * P:(m + 1) * P],
                                rhs=h2[kt][:, :T],
                                start=(kt == 0), stop=(kt == KH - 1))
                        z_m = pa_sb.tile([P, 512], f32r, tag=f"zsb{m}")
                        nc.scalar.activation(z_m[:, :T], ps_z[:, :T], AF.Copy)
                        zsb.append(z_m)

                    # t transposed to columns for all groups of this block
                    ng = T // P
                    ps_tc = pa_ps_rb.tile([P, 8], f32, tag="ps_tc")
                    for g in range(ng):
                        nc.tensor.matmul(ps_tc[:, 2 * g:2 * g + 2],
                                         lhsT=t_row[:, g * P:(g + 1) * P],
                                         rhs=ones_12r[:],
                                         start=True, stop=True)
                    t_cols = pa_sb1.tile([P, 8], bf16, tag="t_cols")
                    nc.vector.tensor_copy(t_cols[:, :2 * ng],
                                          ps_tc[:, :2 * ng])

                    for g in range(T // P):
                        bkt = (s0 + g * P) // P
                        k = next(kk for kk in range(nch)
                                 if bb[kk] <= bkt < bb[kk + 1])
                        rowt = pa_sb.tile([P, R], bf16, tag="rowt")
                        for m in range(KD):
                            ps_tr = pa_ps_tr.tile([P, P], f32, tag="ps_tr")
                            nc.tensor.transpose(
                                ps_tr[:].bitcast(f32r),
                                zsb[m][:, g * P:(g + 1) * P],
                                ident_r[:])
                            nc.scalar.activation(rowt[:, m * P:(m + 1) * P],
                                                 ps_tr[:], AF.Copy)
                            nc.vector.tensor_scalar(
                                out=agg_sb[:, bkt * D + m * P:
                                           bkt * D + (m + 1) * P],
                                in0=ps_tr[:],
                                scalar1=2.0, scalar2=None, op0=ALU.mult)
                        nc.vector.tensor_copy(rowt[:, D:D + 1],
                                              t_cols[:, 2 * g:2 * g + 1])
                        nc.sync.dma_start(
                            ag_in[k][(bkt - bb[k]) * P + (0):
                                     (bkt - bb[k]) * P + P, :], rowt[:])

            # ================== AllGather (chunked) ==================
            for k in range(nch):
                nc.gpsimd.collective_compute(
                    "AllGather", ALU.bypass,
                    replica_groups=[list(range(cfg.n_cores))],
                    ins=[ag_in[k][:, :]],
                    outs=[ag_half[k][:, :]],
                )

            # ======================= Phase B =======================
            C5 = CPB // 2
            with (
                tc.tile_pool(name="pb_g", bufs=cfg.g_bufs) as pb_g,
                tc.tile_pool(name="pb_oh", bufs=3) as pb_oh,
                tc.tile_pool(name="pb_sb", bufs=3) as pb_sb,
                tc.tile_pool(name="pb_ps", bufs=2, space="PSUM") as pb_ps,
            ):
                IW2 = IW // 2
                # zero gather buffers once (stale-but-finite afterwards)
                for i in range(cfg.g_bufs):
                    gz = pb_g.tile([P, CPB, R], bf16, tag="G", name=f"Gz{i}")
                    nc.vector.memset(gz[:], 0.0)

                for half in range(2):
                    for q in range(NP):
                        G = pb_g.tile([P, CPB, R], bf16, tag="G")
                        for j in range(2):
                            b = q * 2 + j
                            nc.gpsimd.dma_gather(
                                out_ap=G[:, j * C5:(j + 1) * C5, :],
                                in_ap=ag_half[half][:, :],
                                idxs_ap=idx_sb[:, (q * 2 + half) * IW + j * IW2:
                                               (q * 2 + half) * IW + (j + 1) * IW2],
                                num_idxs=cfg.half_slots,
                                num_idxs_reg=(cfg.half_slots if cnts is None
                                              else int(cnts[b * 2 + half])),
                                elem_size=R,
                                queue_num=half,
                            )

                        for j in range(2):
                            b = q * 2 + j
                            c0 = (b * CPB + half * C5) * P
                            oh_t = pb_oh.tile([P, C5 * P], bf16, tag="oh")
                            nc.sync.dma_start(oh_t[:], ohT[:, c0:c0 + C5 * P])
                            oh2_t = pb_oh.tile([P, C5 * P], bf16, tag="oh2")
                            nc.sync.dma_start(oh2_t[:],
                                              oh2T[:, c0:c0 + C5 * P])

                            # s[dst] as a column, s_d[slot] via oh2 matmuls,
                            # + t[slot] via identity matmul on the G t-column
                            ps_sc = pb_ps.tile([P, 2], f32, tag="ps_sc")
                            nc.tensor.matmul(ps_sc[:],
                                             lhsT=s_row[:, b * P:(b + 1) * P],
                                             rhs=ones_1bf[:],
                                             start=True, stop=True)
                            s_col = pb_sb.tile([P, 1], bf16, tag="s_col")
                            nc.vector.tensor_copy(s_col[:], ps_sc[:, 0:1])
                            ps_sd = pb_ps.tile([P, C5], f32, tag="ps_sd")
                            for ch in range(C5):
                                nc.tensor.matmul(
                                    ps_sd[:, ch:ch + 1],
                                    lhsT=oh2_t[:, ch * P:(ch + 1) * P],
                                    rhs=s_col[:],
                                    start=True, stop=False)
                                nc.tensor.matmul(
                                    ps_sd[:, ch:ch + 1],
                                    lhsT=ident_bf[:],
                                    rhs=G[:, j * C5 + ch, D:D + 1],
                                    start=False, stop=True)

                            att = pb_sb.tile([P, C5], f32, tag="att")
                            nc.scalar.activation(att[:], ps_sd[:], AF.Sigmoid)

                            # oat = oh * att (per chunk)
                            oat = pb_sb.tile([P, C5 * P], bf16, tag="oat")
                            for ch in range(C5):
                                nc.vector.tensor_scalar(
                                    out=oat[:, ch * P:(ch + 1) * P],
                                    in0=oh_t[:, ch * P:(ch + 1) * P],
                                    scalar1=1.0, scalar2=att[:, ch:ch + 1],
                                    op0=ALU.mult, op1=ALU.mult)

                            ps_agg = pb_ps.tile([P, D], f32, tag="ps_agg")
                            for ch in range(C5):
                                nc.tensor.matmul(
                                    ps_agg[:],
                                    lhsT=oat[:, ch * P:(ch + 1) * P],
                                    rhs=G[:, j * C5 + ch, 0:D],
                                    start=(ch == 0), stop=(ch == C5 - 1))

                            # y += agg_half
                            aslice = agg_sb[:, b * D:(b + 1) * D]
                            nc.vector.tensor_add(aslice, ps_agg[:], aslice)

                            if half == 1:
                                r1 = pb_sb.tile([P, 1], f32, tag="r1")
                                nc.vector.tensor_reduce(r1[:], aslice,
                                                        mybir.AxisListType.X,
                                                        ALU.add)
                                sqd = pb_sb.tile([P, D], bf16, tag="sqd")
                                nc.scalar.activation(sqd[:], aslice, AF.Square)
                                r2 = pb_sb.tile([P, 1], f32, tag="r2")
                                nc.vector.tensor_reduce(r2[:], sqd[:],
                                                        mybir.AxisListType.X,
                                                        ALU.add)
                                mu_b = mu_sb[:, b:b + 1]
                                nc.vector.tensor_scalar(out=mu_b, in0=r1[:],
                                                        scalar1=1.0 / D,
                                                        scalar2=None,
                                                        op0=ALU.mult)
                                mu2 = pb_sb.tile([P, 1], f32, tag="mu2")
                                nc.vector.tensor_mul(mu2[:], mu_b, mu_b)
                                nc.vector.tensor_scalar(
                                    out=var_sb[:, b:b + 1],
                                    in0=r2[:], scalar1=1.0 / D,
                                    scalar2=mu2[:],
                                    op0=ALU.mult, op1=ALU.subtract)

                # ---- pass 2: batched final LN ----
                lnv2 = pb_sb.tile([P, B], f32, tag="lnv2")
                nc.scalar.activation(lnv2[:], var_sb[:], AF.Ln,
                                     bias=eps_col[:, :1])
                rsg = pb_sb.tile([P, B], f32, tag="rsg")
                nc.scalar.activation(rsg[:], lnv2[:], AF.Exp, scale=-0.5)
                mnr = pb_sb.tile([P, B], f32, tag="mnr")
                nc.vector.tensor_mul(mnr[:], mu_sb[:], rsg[:])
                nc.vector.tensor_scalar(out=mnr[:], in0=mnr[:], scalar1=-1.0,
                                        scalar2=None, op0=ALU.mult)
                for b in range(B):
                    on = pb_sb.tile([P, D], f32, tag="on")
                    nc.scalar.activation(on[:], agg_sb[:, b * D:(b + 1) * D],
                                         AF.Identity, bias=mnr[:, b:b + 1],
                                         scale=rsg[:, b:b + 1])
                    nc.sync.dma_start(outN[b * P:(b + 1) * P, :], on[:])

    nc.compile()
    return nc


# ---------------------------------------------------------------------------
# Host-side preparation
# ---------------------------------------------------------------------------

def host_prep(cfg: Cfg, node_ids, edge_index, node_features, emb_table):
    n = node_ids.shape[0]
    S, B, CPB = cfg.shard, cfg.buckets, cfg.cpb
    NCB = cfg.n_cores * B
    row = np.asarray(edge_index[0], np.int64)
    col = np.asarray(edge_index[1], np.int64)
    deg = np.bincount(row, minlength=n)

    order = np.argsort(-deg, kind="stable")
    gb = np.empty(n, np.int64)
    gb[order] = np.arange(n) % NCB

    def slots_for(gb_):
        slot = np.zeros(n, np.int64)
        o2 = np.argsort(gb_, kind="stable")
        gs = gb_[o2]
        start_of = np.searchsorted(gs, np.arange(NCB))
        slot[o2] = np.arange(n) - start_of[gs]
        return slot

    slot_in_b = slots_for(gb)
    assert slot_in_b.max() < P

    bb = np.array(cfg.bucket_base)
    crows = np.array(cfg.chunk_rows)
    crb = np.array(cfg.chunk_row_base)

    def gidx_of(core, s):
        bkt = s // P
        k = np.searchsorted(bb, bkt, side="right") - 1
        return crb[k] + core * crows[k] + (s - bb[k] * P)

    lim = cfg.half_slots
    for it in range(500):
        gsl = gidx_of(gb // B, (gb % B) * P + slot_in_b)
        src_half = (gsl >= cfg.half_rows).astype(np.int64)[col]
        loads = np.zeros((NCB, 2), np.int64)
        np.add.at(loads, (gb[row], src_half), 1)
        over = np.argwhere(loads > lim)
        if len(over) == 0:
            break
        ob, ohalf = over[np.argmax(loads[over[:, 0], over[:, 1]])]
        core = ob // B
        cand_b = np.arange(core * B, (core + 1) * B)
        bn = np.bincount(gb, minlength=NCB)
        mask_e = (gb[row] == ob) & (src_half == ohalf)
        contrib = np.bincount(row[mask_e], minlength=n)
        nodes_in_ob = np.where(gb == ob)[0]
        v = nodes_in_ob[np.argmax(contrib[nodes_in_ob])]
        room = bn[cand_b] < P
        scores = loads[cand_b].max(1).astype(np.float64)
        scores[~room] = np.inf
        scores[cand_b == ob] = np.inf
        tb = cand_b[np.argmin(scores)]
        if not np.isfinite(scores.min()):
            raise RuntimeError("bucket fix-up failed: no room")
        gb[v] = tb
        slot_in_b = slots_for(gb)
    else:
        raise RuntimeError("bucket fix-up did not converge")

    gsl = gidx_of(gb // B, (gb % B) * P + slot_in_b)

    perm = np.full((cfg.n_cores, S), -1, np.int64)
    perm[gb // B, (gb % B) * P + slot_in_b] = np.arange(n)

    e_core = gb[row] // B
    e_b = gb[row] % B
    e_d = slot_in_b[row]
    e_half = (gsl[col] >= cfg.half_rows).astype(np.int64)
    e_gidx = gsl[col] - e_half * cfg.half_rows

    NP_ = B // 2
    IW = (2 * cfg.half_slots) // 16
    dg_all = np.zeros((cfg.n_cores, P, NP_ * 2 * IW), np.int16)
    oh_all = np.zeros((cfg.n_cores, P, B * CPB * P), np.uint16)
    oh2_all = np.zeros((cfg.n_cores, P, B * CPB * P), np.uint16)
    cnt_all = np.zeros((cfg.n_cores, B * 2), np.int64)
    ONE_BF16 = np.uint16(0x3F80)

    key = ((e_core * B + e_b) * 2 + e_half)
    eo = np.argsort(key, kind="stable")
    ks = key[eo]
    bounds = np.searchsorted(ks, np.arange(NCB * 2 + 1))
    for c in range(cfg.n_cores):
        for b in range(B):
            for half in range(2):
                kk = (c * B + b) * 2 + half
                cnt_all[c, b * 2 + half] = bounds[kk + 1] - bounds[kk]
    # num_idxs_reg is baked into the (shared) NEFF, so pad each bucket-half
    # idx list with dummy valid 0-indices up to the cross-core max, with -1
    # sentinels beyond.
    cnts_max = cnt_all.max(axis=0)
    for c in range(cfg.n_cores):
        for b in range(B):
            for half in range(2):
                kk = (c * B + b) * 2 + half
                sel = eo[bounds[kk]:bounds[kk + 1]]
                k = len(sel)
                kp = cfg.half_slots
                km = int(cnts_max[b * 2 + half])
                assert k <= kp, (c, b, half, k)
                idx_pad = np.full(kp, -1, np.int64)
                idx_pad[:k] = e_gidx[sel]
                idx_pad[k:km] = 0
                # merged-pair layout: bucket b occupies the (b%2)-th 640-slot
                # segment of pair q=b//2's idx list for this half
                q, j = divmod(b, 2)
                blk = idx_pad.reshape(kp // 16, 16).T.astype(np.int16)
                off = (q * 2 + half) * IW + j * (kp // 16)
                dg_all[c, :, off:off + kp // 16] = np.tile(blk, (8, 1))
                # one-hot: valid slot i of this half -> (p=i%128,
                # ch=half*5+i//128); oh is slot-major, oh2 dst-major
                i = np.arange(k)
                pp = i % P
                ch = half * (CPB // 2) + i // P
                dd = e_d[sel]
                oh_all[c, pp, (b * CPB + ch) * P + dd] = ONE_BF16
                oh2_all[c, dd, (b * CPB + ch) * P + pp] = ONE_BF16

    # per-core inputs: xT = (features + emb)^T, LN-mean-folded weights
    xT_all = np.zeros((cfg.n_cores, cfg.d_in, S), np.float32)
    nf = np.asarray(node_features, np.float32)
    er = np.asarray(emb_table, np.float32)[np.asarray(node_ids, np.int64)]
    xfull = nf + er
    for c in range(cfg.n_cores):
        pc = perm[c]
        valid = pc >= 0
        xT_all[c][:, valid] = xfull[pc[valid]].T

    return perm, xT_all, dg_all, oh_all, oh2_all, cnts_max


_BUILD_CACHE = {}


def _get_nc(cfg: Cfg, cnts=None):
    key = (cfg, None if cnts is None else tuple(int(x) for x in cnts))
    if key not in _BUILD_CACHE:
        _BUILD_CACHE[key] = build(cfg, cnts)
    return _BUILD_CACHE[key]


def run(cfg: Cfg, node_ids, edge_index, node_features, emb_table,
        W0, b0, g0, be0, W1, b1, g1, be1, W2, b2, g2, be2,
        W3, b3, g3, be3, w_att, b_att):
    import ml_dtypes
    D, H = cfg.d_in, cfg.d_hid
    b_list = [np.asarray(x, np.float32) for x in (b0, b1, b2, b3)]
    g_list = [np.asarray(x, np.float32) for x in (g0, g1, g2, g3)]
    be_list = [np.asarray(x, np.float32) for x in (be0, be1, be2, be3)]
    with_b = any(np.any(x != 0) for x in b_list)
    with_gbe = (any(np.any(x != 1) for x in g_list)
                or any(np.any(x != 0) for x in be_list))
    assert not (with_b or with_gbe), \
        "v3 kernel fast path requires default b/g/be params"
    cfg = replace(cfg, b_att=float(np.asarray(b_att)))

    perm, xT_all, dg_all, oh_all, oh2_all, cnts_max = host_prep(
        cfg, node_ids, edge_index, node_features, emb_table)

    W = [np.asarray(x, np.float32) for x in (W0, W1, W2, W3)]
    # fold the LN mean into W0..W2: W' = W - rowmean-producing rank-1 term
    Wf = [W[i] - (W[i].sum(1) / W[i].shape[1])[:, None] for i in range(3)]
    Wf.append(W[3])
    wa = np.asarray(w_att, np.float32)
    watt2 = np.stack([wa[:H], wa[H:]], axis=1)

    nc = _get_nc(cfg, cnts_max)
    in_maps = []
    for c in range(cfg.n_cores):
        in_maps.append(dict(
            xT=xT_all[c],
            w0=Wf[0], w1=Wf[1], w2=Wf[2], w3=Wf[3], watt=watt2,
            dgidx=dg_all[c],
            ohT=oh_all[c].view(ml_dtypes.bfloat16),
            oh2T=oh2_all[c].view(ml_dtypes.bfloat16),
        ))
    res = run_bass_kernel_spmd(nc, in_maps, core_ids=list(range(cfg.n_cores)),
                               trace=cfg.trace)
    n = node_ids.shape[0]
    out = np.zeros((n, D), np.float32)
    for c in range(cfg.n_cores):
        pc = perm[c]
        valid = pc >= 0
        out[pc[valid]] = res.results[c]["outN"][valid]
    return out, res


def kernel(**inputs) -> np.ndarray:
    out, _ = run(CFG, **inputs)
    return out
